# revision 45
# baseline (speedup 1.0000x reference)
"""Expert-choice MoE layer (NucleusMoELayer) on 8 Trainium2 NeuronCores.

Strategy (expert-parallel):
 - one expert per core; router + gate-normalization replicated from an
   AllGathered logit table; shared expert sharded over tokens (1024/core)
 - router logits from host-transposed hsuT shard (no PE transposes) +
   host-computed timestep bias
 - expert-choice top-1024-per-(batch,expert) via 26-step threshold bisection
   (5-op iterations, fused count via accum_out), woven through the shared
   FFN so its serial latency hides inside the DMA-bound m1 phase
 - shared m1 streams packed (a_i|g_i) weight tiles (8KB resident) so the
   expert W1 can be fully resident (64KB) in parallel; expert W2 aliases
   the shared-W2 tiles after their last use
 - compaction (selected tokens -> dense slots) via per-partition cumsum +
   GpSimd local_scatter of (token-id, gate-hi, gate-lo) uint16 payloads
 - dispatch: indirect-DMA row gather of selected tokens, swiglu FFN in bf16
 - outputs feature-major bf16; host transposes, upcasts, scatter-adds

kernel(**inputs) takes FULL unsharded inputs, returns the FULL output.
"""

import sys

if "/opt/trn_rl_repo" not in sys.path:
    sys.path.insert(0, "/opt/trn_rl_repo")

import numpy as np

import concourse.bacc as bacc
import concourse.bass as bass
import concourse.mybir as mybir
import concourse.tile as tile
from concourse.bass_utils import run_bass_kernel_spmd

dt = mybir.dt
AF = mybir.ActivationFunctionType
ALU = mybir.AluOpType

NCORES = 8
BS, SLEN, DIM = 2, 4096, 1024
INNER = 2048
I2 = 2 * INNER  # 4096
E = 8
CAP = 1024  # tokens per (batch, expert)
T = BS * SLEN  # 8192 global tokens
TSH = T // NCORES  # 1024-token shard per core
SLOTS = BS * CAP  # 2048 routed slots per expert
KD = DIM // 128  # 8 k-chunks over dim
KI = INNER // 128  # 16 k-chunks over inner
BISECT_ITERS = 26


def build_nc():
    nc = bacc.Bacc(None, target_bir_lowering=False, num_devices=NCORES)

    tens = {}

    def din(name, shape, dtype=dt.float32):
        tens[name] = nc.dram_tensor(name, shape, dtype, kind="ExternalInput")

    def dout(name, shape, dtype=dt.float32):
        tens[name] = nc.dram_tensor(name, shape, dtype, kind="ExternalOutput")

    din("hs_b", [T, DIM], dt.bfloat16)
    din("hs_shT_b", [DIM, TSH], dt.bfloat16)
    din("hsuT_sh", [DIM, TSH])
    din("wgt", [DIM, E])
    din("bias_mine", [E, 1])
    din("w1_b", [DIM, I2], dt.bfloat16)  # expert W1, resident
    din("w1p_b", [DIM, I2], dt.bfloat16)  # shared W1, (a_i|g_i)-packed cols
    din("w2_b", [INNER, DIM], dt.bfloat16)  # expert W2
    din("ws2_b", [INNER, DIM], dt.bfloat16)  # shared W2, resident
    din("esel", [128, 16])
    din("dsel", [128, 16])
    din("b2", [16, BS])
    din("lt16", [16, 16])
    din("rsmat", [128, 128])
    din("ones128", [1, 128])
    din("iota_tid", [16, 512], dt.uint16)
    dout("out_routed", [DIM, SLOTS], dt.bfloat16)
    dout("out_idx", [SLOTS, 1], dt.int32)
    dout("out_shared", [DIM, TSH], dt.bfloat16)
    dout("dbg_thr", [128, 1])
    dout("dbg_gate", [BS, CAP])

    with tile.TileContext(nc, num_cores=NCORES) as tc:
        _emit(nc, tc, tens)
    nc.finalize()
    return nc


def _emit(nc, tc, t):
    from contextlib import ExitStack

    ctx = ExitStack()
    with ctx:
        const = ctx.enter_context(tc.tile_pool(name="const", bufs=1))
        sb = ctx.enter_context(tc.tile_pool(name="sb", bufs=2))
        sb1 = ctx.enter_context(tc.tile_pool(name="sb1", bufs=1))
        ws = ctx.enter_context(tc.tile_pool(name="ws", bufs=1))
        hTp = ctx.enter_context(tc.tile_pool(name="hTp", bufs=2))
        cw = ctx.enter_context(tc.tile_pool(name="cw", bufs=1))
        bis = ctx.enter_context(tc.tile_pool(name="bis", bufs=1))
        dr = ctx.enter_context(tc.tile_pool(name="dr", bufs=1, space="DRAM"))
        pmm = ctx.enter_context(tc.tile_pool(name="pmm", bufs=6, space="PSUM"))
        psm = ctx.enter_context(tc.tile_pool(name="psm", bufs=2, space="PSUM"))

        def load_const(name, shape, dtype=dt.float32):
            tl = const.tile(shape, dtype, tag=name)
            nc.sync.dma_start(tl[:], t[name][:])
            return tl

        # shared-expert input: host-transposed shard, straight DMA load
        xsT = []

        def xsT_dma(n):
            xt = sb1.tile([128, KD * 512], dt.bfloat16, tag=f"xT{n}")
            for k in range(KD):
                nc.sync.dma_start(
                    xt[:, k * 512 : (k + 1) * 512],
                    t["hs_shT_b"][k * 128 : (k + 1) * 128, n * 512 : (n + 1) * 512],
                )
            xsT.append(xt)

        # ---------------- units ----------------
        # shared m1, i-outer / chunk-inner, streamed packed weights
        w1s = {}

        def w1s_dma(i):
            tiles = []
            for k in range(KD):
                wt = ws.tile([128, 256], dt.bfloat16, tag=f"w1s_{k}_{i % 2}")
                nc.sync.dma_start(
                    wt[:], t["w1p_b"][k * 128 : (k + 1) * 128, i * 256 : (i + 1) * 256]
                )
                tiles.append(wt)
            w1s[i] = tiles

        # m1-critical first loads ahead of the consts on the DMA rings
        w1s_dma(0)
        xsT_dma(0)

        esel = load_const("esel", [128, 16])
        dsel = load_const("dsel", [128, 16])
        b2 = load_const("b2", [16, BS])
        lt16 = load_const("lt16", [16, 16])
        rsmat = load_const("rsmat", [128, 128])
        ones128 = load_const("ones128", [1, 128])
        iota_tid = load_const("iota_tid", [16, 512], dt.uint16)
        wgt_sb = const.tile([128, KD * E], dt.float32, tag="wgt")
        for k in range(KD):
            nc.sync.dma_start(
                wgt_sb[:, k * E : (k + 1) * E],
                t["wgt"][k * 128 : (k + 1) * 128, :],
            )
        bias_sb = load_const("bias_mine", [E, 1])

        h_sh = []
        for n in range(2):
            h_tile = sb1.tile([128, KI * 512], dt.bfloat16, tag=f"h_sb{n}")
            h_sh.append(h_tile)

        def sh_m1_half(i, n):
            def f():
                if n == 0 and i + 1 < 16:
                    w1s_dma(i + 1)
                wts = w1s[i]
                ps_a = pmm.tile([128, 512], dt.float32, tag="mm")
                for k in range(KD):
                    nc.tensor.matmul(
                        ps_a[:],
                        lhsT=wts[k][:, :128],
                        rhs=xsT[n][:, k * 512 : (k + 1) * 512],
                        start=(k == 0),
                        stop=(k == KD - 1),
                    )
                ps_g = pmm.tile([128, 512], dt.float32, tag="mm")
                for k in range(KD):
                    nc.tensor.matmul(
                        ps_g[:],
                        lhsT=wts[k][:, 128:],
                        rhs=xsT[n][:, k * 512 : (k + 1) * 512],
                        start=(k == 0),
                        stop=(k == KD - 1),
                    )
                sl = sb1.tile([128, 512], dt.bfloat16, tag=f"silu{i % 2}")
                nc.scalar.activation(sl[:], ps_g[:], AF.Silu)
                nc.vector.tensor_mul(
                    h_sh[n][:, i * 512 : (i + 1) * 512], ps_a[:], sl[:]
                )

            return f

        # resident weights: expert W1 (separate), shared W2 (aliased by W2 later)
        wr1 = [None] * KD
        wrs2 = [None] * KI
        wr2 = [None] * KI

        def mk_wload(dst, idx, src, width, tagp):
            def f():
                wt = ws.tile([128, width], dt.bfloat16, tag=f"{tagp}{idx}")
                nc.sync.dma_start(wt[:], src[idx * 128 : (idx + 1) * 128, :])
                dst[idx] = wt

            return f

        # router units
        ag_in = dr.tile([E, TSH], dt.float32)

        def mk_router(n):
            def f():
                lps = psm.tile([128, 512], dt.float32, tag="small")
                for k in range(KD):
                    hT = hTp.tile([128, 512], dt.float32, tag="hT")
                    nc.sync.dma_start(
                        hT[:],
                        t["hsuT_sh"][k * 128 : (k + 1) * 128, n * 512 : (n + 1) * 512],
                    )
                    nc.tensor.matmul(
                        lps[:E, :],
                        lhsT=wgt_sb[:, k * E : (k + 1) * E],
                        rhs=hT[:],
                        start=(k == 0),
                        stop=(k == KD - 1),
                    )
                lchunk = bis.tile([E, 512], dt.float32, tag="lchunk")
                nc.vector.tensor_scalar(
                    lchunk[:], lps[:E, :], bias_sb[:], None, op0=ALU.add
                )
                nc.sync.dma_start(ag_in[:, n * 512 : (n + 1) * 512], lchunk[:])

            return f

        ag_out = dr.tile([NCORES * E, TSH], dt.float32, addr_space="Shared")
        logit_all = sb1.tile([128, 512], dt.float32, tag="logit_all")
        sig = sb1.tile([128, 512], dt.float32, tag="sig")

        def rt_collective():
            nc.gpsimd.collective_compute(
                "AllGather",
                ALU.bypass,
                replica_groups=[list(range(NCORES))],
                ins=[ag_in[:]],
                outs=[ag_out[:]],
            )
            nc.sync.dma_start(
                logit_all[:],
                ag_out[:].rearrange("(r e) (c t) -> (r e c) t", e=E, c=2),
            )

        def rt_sig():
            # woven late so it never head-of-line blocks the Scalar queue
            nc.scalar.activation(sig[:], logit_all[:], AF.Sigmoid)

        # bisection: lo converges to the top-CAP threshold in logit space
        lo = sb1.tile([128, 1], dt.float32, tag="lo")

        def rt_init():
            nc.vector.memset(lo[:], -16.0)

        def mk_bisect(it):
            step = 32.0 / (2.0 ** (it + 1))

            def f():
                mid = bis.tile([128, 1], dt.float32, tag="mid")
                nc.vector.tensor_scalar(mid[:], lo[:], step, None, op0=ALU.add)
                cmp = bis.tile([128, 512], dt.bfloat16, tag="cmp")
                cnt = bis.tile([128, 1], dt.float32, tag="cnt")
                nc.vector.tensor_scalar(
                    cmp[:], logit_all[:], mid[:], 0.0, op0=ALU.is_ge,
                    op1=ALU.add, accum_out=cnt[:],
                )
                cntg_ps = psm.tile([128, 512], dt.float32, tag="small")
                nc.tensor.matmul(
                    cntg_ps[:, :1], lhsT=rsmat[:], rhs=cnt[:], start=True, stop=True
                )
                pred = bis.tile([128, 1], dt.uint8, tag="pred")
                nc.vector.tensor_scalar(
                    pred[:], cntg_ps[:, :1], float(CAP), None, op0=ALU.is_ge
                )
                nc.vector.copy_predicated(lo[:], pred[:], mid[:])

            return f

        gate_t = {}

        def rt_gates():
            nc.sync.dma_start(t["dbg_thr"][:], lo[:])
            mask = sb1.tile([128, 512], dt.float32, tag="gmask")
            nc.vector.tensor_scalar(
                mask[:], logit_all[:], lo[:], None, op0=ALU.is_ge
            )
            g = sig  # in-place: sig is dead after this
            nc.vector.tensor_mul(g[:], sig[:], mask[:])
            gm_ps = psm.tile([128, 512], dt.float32, tag="small")
            nc.tensor.matmul(
                gm_ps[:16, :], lhsT=esel[:], rhs=g[:], start=True, stop=True
            )
            dm_ps = psm.tile([128, 512], dt.float32, tag="small")
            nc.tensor.matmul(
                dm_ps[:16, :], lhsT=dsel[:], rhs=g[:], start=True, stop=True
            )
            dsafe = cw.tile([16, 512], dt.float32, tag="cwa")
            nc.vector.tensor_scalar(
                dsafe[:], dm_ps[:16, :], 1e-12, None, op0=ALU.add
            )
            drec = cw.tile([16, 512], dt.float32, tag="cwb")
            nc.vector.reciprocal_approx_fast(drec[:], dsafe[:])
            ghat_mine = sb1.tile([16, 512], dt.float32, tag="ghat_mine")
            nc.vector.tensor_mul(ghat_mine[:], gm_ps[:16, :], drec[:])
            msk_ps = psm.tile([128, 512], dt.float32, tag="small")
            nc.tensor.matmul(
                msk_ps[:16, :], lhsT=esel[:], rhs=mask[:], start=True, stop=True
            )
            mask_mine = sb1.tile([16, 512], dt.float32, tag="mask_mine")
            nc.vector.tensor_copy(mask_mine[:], msk_ps[:16, :])
            gate_t["ghat_mine"] = ghat_mine
            gate_t["mask_mine"] = mask_mine

        cc = {}
        idx16_buf = dr.tile([SLOTS, 1], dt.int16)
        gate_buf = dr.tile([BS, CAP], dt.float32)

        def rt_compact():
            ghat_mine = gate_t["ghat_mine"]
            mask_mine = gate_t["mask_mine"]
            incl = cw.tile([16, 512], dt.float32, tag="cwf")
            nc.vector.tensor_tensor_scan(
                incl[:], mask_mine[:], mask_mine[:], 0.0,
                op0=ALU.add, op1=ALU.bypass,
            )
            offs_ps = psm.tile([128, 512], dt.float32, tag="small")
            nc.tensor.matmul(
                offs_ps[:16, :1], lhsT=lt16[:], rhs=incl[:, 511:512],
                start=True, stop=True,
            )
            pos = cw.tile([16, 512], dt.float32, tag="cwh")
            nc.vector.tensor_sub(pos[:], incl[:], mask_mine[:])
            offs = cw.tile([16, 1], dt.float32, tag="cwo")
            nc.vector.tensor_copy(offs[:], offs_ps[:16, :1])
            nc.vector.tensor_scalar(pos[:], pos[:], offs[:], None, op0=ALU.add)
            boff = cw.tile([16, 1], dt.float32, tag="cwo2")
            nc.vector.tensor_scalar(
                boff[:], b2[:, 1:2], float(CAP), None, op0=ALU.mult
            )
            nc.vector.tensor_scalar(pos[:], pos[:], boff[:], None, op0=ALU.subtract)
            okm = cw.tile([16, 512], dt.float32, tag="cwa")
            nc.vector.tensor_scalar(
                okm[:], pos[:], float(CAP - 1), None, op0=ALU.is_le
            )
            nc.vector.tensor_mul(okm[:], okm[:], mask_mine[:])
            p1 = cw.tile([16, 512], dt.float32, tag="cwb")
            nc.vector.tensor_scalar(p1[:], pos[:], 1.0, None, op0=ALU.add)
            nc.vector.tensor_mul(p1[:], p1[:], okm[:])
            nc.vector.tensor_scalar(p1[:], p1[:], 1.0, None, op0=ALU.subtract)
            pos_i16 = sb1.tile([16, 512], dt.int16, tag="pos_i16")
            nc.vector.tensor_copy(pos_i16[:], p1[:])

            gbits = (
                ghat_mine[:].bitcast(dt.uint16).rearrange("p (t two) -> p t two", two=2)
            )
            glo = sb1.tile([16, 512], dt.uint16, tag="glo2")
            nc.vector.tensor_copy(glo[:, :, None], gbits[:, :, 0:1])
            ghi = sb1.tile([16, 512], dt.uint16, tag="ghi")
            nc.vector.tensor_copy(ghi[:, :, None], gbits[:, :, 1:2])

            # combined per-batch rows; gates written as interleaved u16 halves
            # (lo, hi) so gf bitcasts straight to the packed f32 gate values
            gf = cw.tile([BS, 2 * CAP], dt.uint16, tag="gf")
            gfw = gf[:].rearrange("b (t two) -> b t two", two=2)
            tid_i = cw.tile([BS, CAP], dt.int32, tag="cwh")
            tid_i16 = cw.tile([BS, CAP], dt.int16, tag="cws3")
            for name, data in (("tid", iota_tid), ("ghi", ghi), ("glo", glo)):
                so = cw.tile([16, CAP], dt.uint16, tag="cws2")
                nc.gpsimd.local_scatter(
                    out_ap=so[:],
                    data_ap=data[:],
                    idxs_ap=pos_i16[:],
                    channels=16,
                    num_elems=CAP,
                    num_idxs=512,
                )
                sf = cw.tile([16, CAP], dt.float32, tag="cwf")
                nc.vector.tensor_copy(sf[:], so[:])
                for h in range(2):
                    cps = psm.tile([128, 512], dt.float32, tag="small")
                    nc.tensor.matmul(
                        cps[:BS, :],
                        lhsT=b2[:],
                        rhs=sf[:, h * 512 : (h + 1) * 512],
                        start=True,
                        stop=True,
                    )
                    hs = slice(h * 512, (h + 1) * 512)
                    if name == "tid":
                        nc.vector.tensor_copy(tid_i[:, hs], cps[:BS, :])
                        nc.vector.tensor_copy(tid_i16[:, hs], cps[:BS, :])
                    elif name == "ghi":
                        nc.vector.tensor_copy(gfw[:, hs, 1:2], cps[:BS, :, None])
                    else:
                        nc.vector.tensor_copy(gfw[:, hs, 0:1], cps[:BS, :, None])

            gatec = gf[:].bitcast(dt.float32)
            nc.sync.dma_start(t["dbg_gate"][:], gatec)
            nc.sync.dma_start(gate_buf[:], gatec)
            nc.sync.dma_start(
                t["out_idx"][:].rearrange("(b t) one -> b (t one)", b=BS), tid_i[:]
            )
            nc.sync.dma_start(
                idx16_buf[:].rearrange("(b t) one -> b (t one)", b=BS), tid_i16[:]
            )

        xe_t = {}
        idx16_w = idx16_buf[:].rearrange("(n c p) one -> p (n c one)", p=16, c=32)

        def mk_dispatch(n):
            def f():
                idxw = sb.tile([128, 32], dt.int16, tag="idxw")
                for rep in range(8):
                    nc.sync.dma_start(
                        idxw[rep * 16 : (rep + 1) * 16, :],
                        idx16_w[:, n * 32 : (n + 1) * 32],
                    )
                xT = sb1.tile([128, KD * 512], dt.bfloat16, tag=f"xT{n % 2}")
                nc.gpsimd.dma_gather(
                    out_ap=xT[:].rearrange("p (k t) -> p k t", t=512),
                    in_ap=t["hs_b"][:],
                    idxs_ap=idxw[:],
                    num_idxs=512,
                    num_idxs_reg=512,
                    elem_size=DIM,
                    transpose=True,
                )
                xe_t[n] = xT
                grow = sb.tile([1, 512], dt.float32, tag="grow")
                nc.sync.dma_start(
                    grow[:],
                    gate_buf[:].rearrange("b (m t) -> (b m) t", t=512)[n : n + 1, :],
                )
                grep_ps = psm.tile([128, 512], dt.float32, tag="small")
                nc.tensor.matmul(
                    grep_ps[:], lhsT=ones128[:], rhs=grow[:], start=True, stop=True
                )
                gsb = sb1.tile([128, 512], dt.float32, tag=f"gate{n % 2}")
                nc.vector.tensor_copy(gsb[:], grep_ps[:])
                gate_t[f"g{n}"] = gsb

            return f

        # expert FFN units (resident wr1 / wr2)
        def mk_m1(n, i, h_sb):
            def f():
                xT = xe_t[n]
                ps_a = pmm.tile([128, 512], dt.float32, tag="mm")
                for k in range(KD):
                    nc.tensor.matmul(
                        ps_a[:],
                        lhsT=wr1[k][:, i * 128 : (i + 1) * 128],
                        rhs=xT[:, k * 512 : (k + 1) * 512],
                        start=(k == 0),
                        stop=(k == KD - 1),
                    )
                ps_g = pmm.tile([128, 512], dt.float32, tag="mm")
                for k in range(KD):
                    nc.tensor.matmul(
                        ps_g[:],
                        lhsT=wr1[k][:, (16 + i) * 128 : (17 + i) * 128],
                        rhs=xT[:, k * 512 : (k + 1) * 512],
                        start=(k == 0),
                        stop=(k == KD - 1),
                    )
                sl = sb1.tile([128, 512], dt.bfloat16, tag=f"silu{i % 2}")
                nc.scalar.activation(sl[:], ps_g[:], AF.Silu)
                nc.vector.tensor_mul(
                    h_sb[:, i * 512 : (i + 1) * 512], ps_a[:], sl[:]
                )

            return f

        def mk_m2(wr2_l, h_sb, out_dram, out_col, gate_sb, mo):
            def f():
                ps2 = pmm.tile([128, 512], dt.float32, tag="mm")
                for k2 in range(KI):
                    nc.tensor.matmul(
                        ps2[:],
                        lhsT=wr2_l[k2][:, mo * 128 : (mo + 1) * 128],
                        rhs=h_sb[:, k2 * 512 : (k2 + 1) * 512],
                        start=(k2 == 0),
                        stop=(k2 == KI - 1),
                    )
                yo = sb1.tile([128, 512], dt.bfloat16, tag=f"yo{mo % 2}")
                if gate_sb is not None:
                    nc.vector.tensor_mul(yo[:], ps2[:], gate_sb[:])
                else:
                    nc.scalar.activation(yo[:], ps2[:], AF.Copy)
                nc.sync.dma_start(
                    out_dram[mo * 128 : (mo + 1) * 128, out_col : out_col + 512],
                    yo[:],
                )

            return f

        # ---------------- emission schedule ----------------
        # routing units, in dependency order; dispatch 0/1 right after
        # compact so their ring positions precede all later DMA traffic
        routing_units = [rt_init, rt_sig]
        for it in range(BISECT_ITERS):
            routing_units.append(mk_bisect(it))
        routing_units.append(rt_gates)
        routing_units.append(rt_compact)
        routing_units.append(mk_dispatch(0))
        routing_units.append(mk_dispatch(1))

        # shared W2 woven into phase 1 (needed when m1 ends ~100us); expert W1
        # waits for phase 2, keeping 8.4MB out of the phase-1 DMA path
        wload_units = [
            mk_wload(wrs2, k2, t["ws2_b"], DIM, "wr2s_") for k2 in range(KI)
        ]

        # phase 1: shared m1 (32 half-units); input/router DMAs first, the
        # routing chain woven 3 per half-unit from half-unit 6 (its stalls
        # hide inside the DMA-bound m1 pace)
        mk_router(0)()
        xsT_dma(1)
        mk_router(1)()
        rt_collective()

        ru = iter(routing_units)
        wl = iter(wload_units)
        half_units = []
        for i in range(16):
            half_units.append(sh_m1_half(i, 0))
            half_units.append(sh_m1_half(i, 1))
        emitted = 0
        for ui, u in enumerate(half_units):
            u()
            nxt_w = next(wl, None)
            if nxt_w is not None:
                nxt_w()
            if ui >= 6:
                want = (ui - 5) * 3
                while emitted < want:
                    nxt = next(ru, None)
                    if nxt is None:
                        break
                    nxt()
                    emitted += 1
        for r in ru:
            r()
        for w in wl:
            w()

        # phase 2: shared m2 batch 0 only (8 units) with expert W1 loads
        # woven in; batch 1 is deferred into the expert phase so the PE
        # reaches the expert chunks ~35us earlier and batch-1's output
        # writes leave the pre-gather fence window
        wl1 = iter(
            [mk_wload(wr1, k, t["w1_b"], I2, "wr1e_") for k in range(KD)]
        )
        for mo in range(KD):
            mk_m2(wrs2, h_sh[0], t["out_shared"], 0, None, mo)()
            nxt_w = next(wl1, None)
            if nxt_w is not None:
                nxt_w()

        # phase 3: expert FFN, 4 chunks of 512 slots. Expert W2 (aliasing the
        # wrs2 tiles) is woven into chunk 0's m1 so its in-flight window sits
        # in an otherwise DMA-quiet stretch, after the dispatch gathers.
        h_e = []
        for n in range(SLOTS // 512):
            hnew = sb1.tile([128, KI * 512], dt.bfloat16, tag=f"h_sb{n % 2}")
            h_e.append(hnew)
            for i in range(16):
                mk_m1(n, i, hnew)()
            if n == 0:
                # shared m2 batch 1 must fully read the wrs2 tiles before
                # the expert W2 loads overwrite them; chunk 0's m2 is then
                # deferred past chunk 1's m1 so the W2 DMA (4.2MB) has PE
                # cover to land instead of stalling m2's k2 stream
                for mo in range(KD):
                    mk_m2(wrs2, h_sh[1], t["out_shared"], 512, None, mo)()
                for k2 in range(KI):
                    mk_wload(wr2, k2, t["w2_b"], DIM, "wr2s_")()
                mk_dispatch(2)()
                continue
            if n == 1:
                for mo in range(KD):
                    mk_m2(wr2, h_e[0], t["out_routed"], 0, gate_t["g0"], mo)()
            for mo in range(KD):
                mk_m2(wr2, hnew, t["out_routed"], n * 512, gate_t[f"g{n}"], mo)()
            if n + 2 < SLOTS // 512:
                mk_dispatch(n + 2)()


# ======================= host side =======================

_CACHED_NC = None


def _get_nc():
    global _CACHED_NC
    if _CACHED_NC is None:
        _CACHED_NC = build_nc()
    return _CACHED_NC


def make_in_maps(inputs):
    hs_flat = np.ascontiguousarray(
        np.asarray(inputs["hidden_states"], dtype=np.float32).reshape(T, DIM)
    )
    hsu_flat = np.ascontiguousarray(
        np.asarray(inputs["hidden_states_unmodulated"], dtype=np.float32).reshape(
            T, DIM
        )
    )
    ts = np.asarray(inputs["timestep"], dtype=np.float32)
    Wg = np.asarray(inputs["Wg"], dtype=np.float32)
    W1 = np.asarray(inputs["W1"], dtype=np.float32)
    W2 = np.asarray(inputs["W2"], dtype=np.float32)
    Ws1 = np.asarray(inputs["Ws1"], dtype=np.float32)
    Ws2 = np.asarray(inputs["Ws2"], dtype=np.float32)

    lt16 = np.triu(np.ones((16, 16), np.float32), 1)  # lhsT[k,m]=1 iff k<m
    b2 = np.zeros((16, BS), np.float32)
    b2[:8, 0] = 1.0
    b2[8:, 1] = 1.0
    # partition layout: p = r*16 + e*2 + c  (r = source core, e = expert,
    # c = 512-token half of the core's shard)
    p = np.arange(128)
    pb = p // 64  # batch  (r//4)
    pe = (p % 16) // 2  # expert
    rsmat = ((pb[:, None] == pb[None, :]) & (pe[:, None] == pe[None, :])).astype(
        np.float32
    )
    # dsel[p, j]: p belongs to token-group j = r*2 + c (sum over experts)
    j = np.arange(16)
    dsel = ((p[:, None] // 16 == j[None, :] // 2) & (p[:, None] % 2 == j[None, :] % 2)
            ).astype(np.float32)
    ones128 = np.ones((1, 128), np.float32)
    jj = np.arange(16)[:, None]
    tt = np.arange(512)[None, :]
    iota_tid = (jj * 512 + tt).astype(np.uint16)
    # router: transposed Wg (hsu half) and host-computed timestep bias
    wgt = np.ascontiguousarray(Wg[:, DIM:].T)  # [DIM, E]
    bias_all = ts @ Wg[:, :DIM].T  # [BS, E]

    import ml_dtypes

    bf16 = ml_dtypes.bfloat16
    hs_b = hs_flat.astype(bf16)
    W1_b = W1.astype(bf16)
    W2_b = W2.astype(bf16)
    # shared W1 packed (a_i | g_i) column pairs
    a = Ws1[:, :INNER].reshape(DIM, 16, 128)
    g = Ws1[:, INNER:].reshape(DIM, 16, 128)
    Ws1p = np.ascontiguousarray(
        np.concatenate([a[:, :, None, :], g[:, :, None, :]], axis=2).reshape(DIM, I2)
    ).astype(bf16)
    Ws2_b = np.ascontiguousarray(Ws2.astype(bf16))
    in_maps = []
    for c in range(NCORES):
        # extract my expert's 16 rows in (b-major, chunk) order:
        # j = r*2 + cc  ->  partition (j//2)*16 + c*2 + (j%2)
        esel = np.zeros((128, 16), np.float32)
        for jx in range(16):
            esel[(jx // 2) * 16 + c * 2 + (jx % 2), jx] = 1.0
        in_maps.append(
            {
                "hs_b": hs_b,
                "hs_shT_b": np.ascontiguousarray(hs_b[c * TSH : (c + 1) * TSH].T),
                "hsuT_sh": np.ascontiguousarray(
                    hsu_flat[c * TSH : (c + 1) * TSH].T
                ),
                "wgt": wgt,
                "bias_mine": np.ascontiguousarray(
                    bias_all[c // 4].reshape(E, 1)
                ),
                "w1_b": np.ascontiguousarray(W1_b[c]),
                "w1p_b": Ws1p,
                "w2_b": np.ascontiguousarray(W2_b[c]),
                "ws2_b": Ws2_b,
                "esel": esel,
                "dsel": dsel,
                "b2": b2,
                "lt16": lt16,
                "rsmat": rsmat,
                "ones128": ones128,
                "iota_tid": iota_tid,
            }
        )
    return in_maps


def combine(results):
    out = np.empty((T, DIM), np.float32)
    for c in range(NCORES):
        out[c * TSH : (c + 1) * TSH] = results[c]["out_shared"].T.astype(np.float32)
    for c in range(NCORES):
        idx = results[c]["out_idx"].reshape(SLOTS)
        out[idx] += results[c]["out_routed"].T.astype(np.float32)
    return out.reshape(BS, SLEN, DIM)


def kernel(**inputs):
    nc = _get_nc()
    in_maps = make_in_maps(inputs)
    res = run_bass_kernel_spmd(nc, in_maps, list(range(NCORES))).results
    return combine(res)


if __name__ == "__main__":
    nc = build_nc()
    print("build ok:", len(nc.inst_map), "instructions")


# revision 46
# speedup vs baseline: 1.0202x; 1.0202x over previous
"""Expert-choice MoE layer (NucleusMoELayer) on 8 Trainium2 NeuronCores.

Strategy (expert-parallel):
 - one expert per core; router + gate-normalization replicated from an
   AllGathered logit table; shared expert sharded over tokens (1024/core)
 - router logits from host-transposed hsuT shard (no PE transposes) +
   host-computed timestep bias
 - expert-choice top-1024-per-(batch,expert) via 26-step threshold bisection
   (5-op iterations, fused count via accum_out), woven through the shared
   FFN so its serial latency hides inside the DMA-bound m1 phase
 - shared m1 streams packed (a_i|g_i) weight tiles (8KB resident) so the
   expert W1 can be fully resident (64KB) in parallel; expert W2 aliases
   the shared-W2 tiles after their last use
 - compaction (selected tokens -> dense slots) via per-partition cumsum +
   GpSimd local_scatter of (token-id, gate-hi, gate-lo) uint16 payloads
 - dispatch: indirect-DMA row gather of selected tokens, swiglu FFN in bf16
 - outputs feature-major bf16; host transposes, upcasts, scatter-adds

kernel(**inputs) takes FULL unsharded inputs, returns the FULL output.
"""

import sys

if "/opt/trn_rl_repo" not in sys.path:
    sys.path.insert(0, "/opt/trn_rl_repo")

import numpy as np

import concourse.bacc as bacc
import concourse.bass as bass
import concourse.mybir as mybir
import concourse.tile as tile
from concourse.bass_utils import run_bass_kernel_spmd

dt = mybir.dt
AF = mybir.ActivationFunctionType
ALU = mybir.AluOpType

NCORES = 8
BS, SLEN, DIM = 2, 4096, 1024
INNER = 2048
I2 = 2 * INNER  # 4096
E = 8
CAP = 1024  # tokens per (batch, expert)
T = BS * SLEN  # 8192 global tokens
TSH = T // NCORES  # 1024-token shard per core
SLOTS = BS * CAP  # 2048 routed slots per expert
KD = DIM // 128  # 8 k-chunks over dim
KI = INNER // 128  # 16 k-chunks over inner
BISECT_ITERS = 26


def build_nc():
    nc = bacc.Bacc(None, target_bir_lowering=False, num_devices=NCORES)

    tens = {}

    def din(name, shape, dtype=dt.float32):
        tens[name] = nc.dram_tensor(name, shape, dtype, kind="ExternalInput")

    def dout(name, shape, dtype=dt.float32):
        tens[name] = nc.dram_tensor(name, shape, dtype, kind="ExternalOutput")

    din("hs_b", [T, DIM], dt.bfloat16)
    din("hs_shT_b", [DIM, TSH], dt.bfloat16)
    din("hsuT_sh", [DIM, TSH])
    din("wgt", [DIM, E])
    din("bias_mine", [E, 1])
    din("w1_b", [DIM, I2], dt.bfloat16)  # expert W1, resident
    din("w1p_b", [DIM, I2], dt.bfloat16)  # shared W1, (a_i|g_i)-packed cols
    din("w2_b", [INNER, DIM], dt.bfloat16)  # expert W2
    din("ws2_b", [INNER, DIM], dt.bfloat16)  # shared W2, resident
    din("esel", [128, 16])
    din("dsel", [128, 16])
    din("b2", [16, BS])
    din("lt16", [16, 16])
    din("rsmat", [128, 128])
    din("ones128", [1, 128])
    din("iota_tid", [16, 512], dt.uint16)
    dout("out_routed", [DIM, SLOTS], dt.bfloat16)
    dout("out_idx", [SLOTS, 1], dt.int32)
    dout("out_shared", [DIM, TSH], dt.bfloat16)
    dout("dbg_thr", [128, 1])
    dout("dbg_gate", [BS, CAP])

    with tile.TileContext(nc, num_cores=NCORES) as tc:
        _emit(nc, tc, tens)
    nc.finalize()
    return nc


def _emit(nc, tc, t):
    from contextlib import ExitStack

    ctx = ExitStack()
    with ctx:
        const = ctx.enter_context(tc.tile_pool(name="const", bufs=1))
        sb = ctx.enter_context(tc.tile_pool(name="sb", bufs=2))
        sb1 = ctx.enter_context(tc.tile_pool(name="sb1", bufs=1))
        ws = ctx.enter_context(tc.tile_pool(name="ws", bufs=1))
        hTp = ctx.enter_context(tc.tile_pool(name="hTp", bufs=2))
        cw = ctx.enter_context(tc.tile_pool(name="cw", bufs=1))
        bis = ctx.enter_context(tc.tile_pool(name="bis", bufs=1))
        dr = ctx.enter_context(tc.tile_pool(name="dr", bufs=1, space="DRAM"))
        pmm = ctx.enter_context(tc.tile_pool(name="pmm", bufs=6, space="PSUM"))
        psm = ctx.enter_context(tc.tile_pool(name="psm", bufs=2, space="PSUM"))

        def load_const(name, shape, dtype=dt.float32):
            tl = const.tile(shape, dtype, tag=name)
            nc.sync.dma_start(tl[:], t[name][:])
            return tl

        esel = load_const("esel", [128, 16])
        dsel = load_const("dsel", [128, 16])
        b2 = load_const("b2", [16, BS])
        lt16 = load_const("lt16", [16, 16])
        rsmat = load_const("rsmat", [128, 128])
        ones128 = load_const("ones128", [1, 128])
        iota_tid = load_const("iota_tid", [16, 512], dt.uint16)
        wgt_sb = const.tile([128, KD * E], dt.float32, tag="wgt")
        for k in range(KD):
            nc.sync.dma_start(
                wgt_sb[:, k * E : (k + 1) * E],
                t["wgt"][k * 128 : (k + 1) * 128, :],
            )
        bias_sb = load_const("bias_mine", [E, 1])

        # shared-expert input: host-transposed shard, straight DMA load
        xsT = []

        def xsT_dma(n):
            xt = sb1.tile([128, KD * 512], dt.bfloat16, tag=f"xT{n}")
            for k in range(KD):
                nc.sync.dma_start(
                    xt[:, k * 512 : (k + 1) * 512],
                    t["hs_shT_b"][k * 128 : (k + 1) * 128, n * 512 : (n + 1) * 512],
                )
            xsT.append(xt)

        # ---------------- units ----------------
        # shared m1, i-outer / chunk-inner, streamed packed weights
        w1s = {}

        def w1s_dma(i):
            tiles = []
            for k in range(KD):
                wt = ws.tile([128, 256], dt.bfloat16, tag=f"w1s_{k}_{i % 2}")
                nc.sync.dma_start(
                    wt[:], t["w1p_b"][k * 128 : (k + 1) * 128, i * 256 : (i + 1) * 256]
                )
                tiles.append(wt)
            w1s[i] = tiles

        h_sh = []
        for n in range(2):
            h_tile = sb1.tile([128, KI * 512], dt.bfloat16, tag=f"h_sb{n}")
            h_sh.append(h_tile)

        def sh_m1_half(i, n):
            def f():
                if n == 0 and i + 1 < 16:
                    w1s_dma(i + 1)
                wts = w1s[i]
                ps_a = pmm.tile([128, 512], dt.float32, tag="mm")
                for k in range(KD):
                    nc.tensor.matmul(
                        ps_a[:],
                        lhsT=wts[k][:, :128],
                        rhs=xsT[n][:, k * 512 : (k + 1) * 512],
                        start=(k == 0),
                        stop=(k == KD - 1),
                    )
                ps_g = pmm.tile([128, 512], dt.float32, tag="mm")
                for k in range(KD):
                    nc.tensor.matmul(
                        ps_g[:],
                        lhsT=wts[k][:, 128:],
                        rhs=xsT[n][:, k * 512 : (k + 1) * 512],
                        start=(k == 0),
                        stop=(k == KD - 1),
                    )
                sl = sb1.tile([128, 512], dt.bfloat16, tag=f"silu{i % 2}")
                nc.scalar.activation(sl[:], ps_g[:], AF.Silu)
                nc.vector.tensor_mul(
                    h_sh[n][:, i * 512 : (i + 1) * 512], ps_a[:], sl[:]
                )

            return f

        # resident weights: expert W1 (separate), shared W2 (aliased by W2 later)
        wr1 = [None] * KD
        wrs2 = [None] * KI
        wr2 = [None] * KI

        def mk_wload(dst, idx, src, width, tagp):
            def f():
                wt = ws.tile([128, width], dt.bfloat16, tag=f"{tagp}{idx}")
                nc.sync.dma_start(wt[:], src[idx * 128 : (idx + 1) * 128, :])
                dst[idx] = wt

            return f

        # router units
        ag_in = dr.tile([E, TSH], dt.float32)

        def mk_router(n):
            def f():
                lps = psm.tile([128, 512], dt.float32, tag="small")
                for k in range(KD):
                    hT = hTp.tile([128, 512], dt.float32, tag="hT")
                    nc.sync.dma_start(
                        hT[:],
                        t["hsuT_sh"][k * 128 : (k + 1) * 128, n * 512 : (n + 1) * 512],
                    )
                    nc.tensor.matmul(
                        lps[:E, :],
                        lhsT=wgt_sb[:, k * E : (k + 1) * E],
                        rhs=hT[:],
                        start=(k == 0),
                        stop=(k == KD - 1),
                    )
                lchunk = bis.tile([E, 512], dt.float32, tag="lchunk")
                nc.vector.tensor_scalar(
                    lchunk[:], lps[:E, :], bias_sb[:], None, op0=ALU.add
                )
                nc.sync.dma_start(ag_in[:, n * 512 : (n + 1) * 512], lchunk[:])

            return f

        ag_out = dr.tile([NCORES * E, TSH], dt.float32, addr_space="Shared")
        logit_all = sb1.tile([128, 512], dt.float32, tag="logit_all")
        sig = sb1.tile([128, 512], dt.float32, tag="sig")

        def rt_collective():
            nc.gpsimd.collective_compute(
                "AllGather",
                ALU.bypass,
                replica_groups=[list(range(NCORES))],
                ins=[ag_in[:]],
                outs=[ag_out[:]],
            )
            nc.sync.dma_start(
                logit_all[:],
                ag_out[:].rearrange("(r e) (c t) -> (r e c) t", e=E, c=2),
            )

        def rt_sig():
            # woven late so it never head-of-line blocks the Scalar queue
            nc.scalar.activation(sig[:], logit_all[:], AF.Sigmoid)

        # bisection: lo converges to the top-CAP threshold in logit space
        lo = sb1.tile([128, 1], dt.float32, tag="lo")

        def rt_init():
            nc.vector.memset(lo[:], -16.0)

        def mk_bisect(it):
            step = 32.0 / (2.0 ** (it + 1))

            def f():
                mid = bis.tile([128, 1], dt.float32, tag="mid")
                nc.vector.tensor_scalar(mid[:], lo[:], step, None, op0=ALU.add)
                cmp = bis.tile([128, 512], dt.bfloat16, tag="cmp")
                cnt = bis.tile([128, 1], dt.float32, tag="cnt")
                nc.vector.tensor_scalar(
                    cmp[:], logit_all[:], mid[:], 0.0, op0=ALU.is_ge,
                    op1=ALU.add, accum_out=cnt[:],
                )
                cntg_ps = psm.tile([128, 512], dt.float32, tag="small")
                nc.tensor.matmul(
                    cntg_ps[:, :1], lhsT=rsmat[:], rhs=cnt[:], start=True, stop=True
                )
                pred = bis.tile([128, 1], dt.uint8, tag="pred")
                nc.vector.tensor_scalar(
                    pred[:], cntg_ps[:, :1], float(CAP), None, op0=ALU.is_ge
                )
                nc.vector.copy_predicated(lo[:], pred[:], mid[:])

            return f

        gate_t = {}

        def rt_gates():
            nc.sync.dma_start(t["dbg_thr"][:], lo[:])
            mask = sb1.tile([128, 512], dt.float32, tag="gmask")
            nc.vector.tensor_scalar(
                mask[:], logit_all[:], lo[:], None, op0=ALU.is_ge
            )
            g = sig  # in-place: sig is dead after this
            nc.vector.tensor_mul(g[:], sig[:], mask[:])
            gm_ps = psm.tile([128, 512], dt.float32, tag="small")
            nc.tensor.matmul(
                gm_ps[:16, :], lhsT=esel[:], rhs=g[:], start=True, stop=True
            )
            dm_ps = psm.tile([128, 512], dt.float32, tag="small")
            nc.tensor.matmul(
                dm_ps[:16, :], lhsT=dsel[:], rhs=g[:], start=True, stop=True
            )
            dsafe = cw.tile([16, 512], dt.float32, tag="cwa")
            nc.vector.tensor_scalar(
                dsafe[:], dm_ps[:16, :], 1e-12, None, op0=ALU.add
            )
            drec = cw.tile([16, 512], dt.float32, tag="cwb")
            nc.vector.reciprocal_approx_fast(drec[:], dsafe[:])
            ghat_mine = sb1.tile([16, 512], dt.float32, tag="ghat_mine")
            nc.vector.tensor_mul(ghat_mine[:], gm_ps[:16, :], drec[:])
            msk_ps = psm.tile([128, 512], dt.float32, tag="small")
            nc.tensor.matmul(
                msk_ps[:16, :], lhsT=esel[:], rhs=mask[:], start=True, stop=True
            )
            mask_mine = sb1.tile([16, 512], dt.float32, tag="mask_mine")
            nc.vector.tensor_copy(mask_mine[:], msk_ps[:16, :])
            gate_t["ghat_mine"] = ghat_mine
            gate_t["mask_mine"] = mask_mine

        cc = {}
        idx16_buf = dr.tile([SLOTS, 1], dt.int16)
        gate_buf = dr.tile([BS, CAP], dt.float32)

        def rt_compact():
            ghat_mine = gate_t["ghat_mine"]
            mask_mine = gate_t["mask_mine"]
            incl = cw.tile([16, 512], dt.float32, tag="cwf")
            nc.vector.tensor_tensor_scan(
                incl[:], mask_mine[:], mask_mine[:], 0.0,
                op0=ALU.add, op1=ALU.bypass,
            )
            offs_ps = psm.tile([128, 512], dt.float32, tag="small")
            nc.tensor.matmul(
                offs_ps[:16, :1], lhsT=lt16[:], rhs=incl[:, 511:512],
                start=True, stop=True,
            )
            pos = cw.tile([16, 512], dt.float32, tag="cwh")
            nc.vector.tensor_sub(pos[:], incl[:], mask_mine[:])
            offs = cw.tile([16, 1], dt.float32, tag="cwo")
            nc.vector.tensor_copy(offs[:], offs_ps[:16, :1])
            nc.vector.tensor_scalar(pos[:], pos[:], offs[:], None, op0=ALU.add)
            boff = cw.tile([16, 1], dt.float32, tag="cwo2")
            nc.vector.tensor_scalar(
                boff[:], b2[:, 1:2], float(CAP), None, op0=ALU.mult
            )
            nc.vector.tensor_scalar(pos[:], pos[:], boff[:], None, op0=ALU.subtract)
            okm = cw.tile([16, 512], dt.float32, tag="cwa")
            nc.vector.tensor_scalar(
                okm[:], pos[:], float(CAP - 1), None, op0=ALU.is_le
            )
            nc.vector.tensor_mul(okm[:], okm[:], mask_mine[:])
            p1 = cw.tile([16, 512], dt.float32, tag="cwb")
            nc.vector.tensor_scalar(p1[:], pos[:], 1.0, None, op0=ALU.add)
            nc.vector.tensor_mul(p1[:], p1[:], okm[:])
            nc.vector.tensor_scalar(p1[:], p1[:], 1.0, None, op0=ALU.subtract)
            pos_i16 = sb1.tile([16, 512], dt.int16, tag="pos_i16")
            nc.vector.tensor_copy(pos_i16[:], p1[:])

            gbits = (
                ghat_mine[:].bitcast(dt.uint16).rearrange("p (t two) -> p t two", two=2)
            )
            glo = sb1.tile([16, 512], dt.uint16, tag="glo2")
            nc.vector.tensor_copy(glo[:, :, None], gbits[:, :, 0:1])
            ghi = sb1.tile([16, 512], dt.uint16, tag="ghi")
            nc.vector.tensor_copy(ghi[:, :, None], gbits[:, :, 1:2])

            # combined per-batch rows; gates written as interleaved u16 halves
            # (lo, hi) so gf bitcasts straight to the packed f32 gate values
            gf = cw.tile([BS, 2 * CAP], dt.uint16, tag="gf")
            gfw = gf[:].rearrange("b (t two) -> b t two", two=2)
            tid_i = cw.tile([BS, CAP], dt.int32, tag="cwh")
            tid_i16 = cw.tile([BS, CAP], dt.int16, tag="cws3")
            for name, data in (("tid", iota_tid), ("ghi", ghi), ("glo", glo)):
                so = cw.tile([16, CAP], dt.uint16, tag="cws2")
                nc.gpsimd.local_scatter(
                    out_ap=so[:],
                    data_ap=data[:],
                    idxs_ap=pos_i16[:],
                    channels=16,
                    num_elems=CAP,
                    num_idxs=512,
                )
                sf = cw.tile([16, CAP], dt.float32, tag="cwf")
                nc.vector.tensor_copy(sf[:], so[:])
                for h in range(2):
                    cps = psm.tile([128, 512], dt.float32, tag="small")
                    nc.tensor.matmul(
                        cps[:BS, :],
                        lhsT=b2[:],
                        rhs=sf[:, h * 512 : (h + 1) * 512],
                        start=True,
                        stop=True,
                    )
                    hs = slice(h * 512, (h + 1) * 512)
                    if name == "tid":
                        nc.vector.tensor_copy(tid_i[:, hs], cps[:BS, :])
                        nc.vector.tensor_copy(tid_i16[:, hs], cps[:BS, :])
                    elif name == "ghi":
                        nc.vector.tensor_copy(gfw[:, hs, 1:2], cps[:BS, :, None])
                    else:
                        nc.vector.tensor_copy(gfw[:, hs, 0:1], cps[:BS, :, None])

            gatec = gf[:].bitcast(dt.float32)
            nc.sync.dma_start(t["dbg_gate"][:], gatec)
            nc.sync.dma_start(gate_buf[:], gatec)
            nc.sync.dma_start(
                t["out_idx"][:].rearrange("(b t) one -> b (t one)", b=BS), tid_i[:]
            )
            nc.sync.dma_start(
                idx16_buf[:].rearrange("(b t) one -> b (t one)", b=BS), tid_i16[:]
            )

        xe_t = {}
        idx16_w = idx16_buf[:].rearrange("(n c p) one -> p (n c one)", p=16, c=32)

        def mk_dispatch(n):
            def f():
                idxw = sb.tile([128, 32], dt.int16, tag="idxw")
                for rep in range(8):
                    nc.sync.dma_start(
                        idxw[rep * 16 : (rep + 1) * 16, :],
                        idx16_w[:, n * 32 : (n + 1) * 32],
                    )
                xT = sb1.tile([128, KD * 512], dt.bfloat16, tag=f"xT{n % 2}")
                nc.gpsimd.dma_gather(
                    out_ap=xT[:].rearrange("p (k t) -> p k t", t=512),
                    in_ap=t["hs_b"][:],
                    idxs_ap=idxw[:],
                    num_idxs=512,
                    num_idxs_reg=512,
                    elem_size=DIM,
                    transpose=True,
                )
                xe_t[n] = xT
                grow = sb.tile([1, 512], dt.float32, tag="grow")
                nc.sync.dma_start(
                    grow[:],
                    gate_buf[:].rearrange("b (m t) -> (b m) t", t=512)[n : n + 1, :],
                )
                grep_ps = psm.tile([128, 512], dt.float32, tag="small")
                nc.tensor.matmul(
                    grep_ps[:], lhsT=ones128[:], rhs=grow[:], start=True, stop=True
                )
                gsb = sb1.tile([128, 512], dt.float32, tag=f"gate{n % 2}")
                nc.vector.tensor_copy(gsb[:], grep_ps[:])
                gate_t[f"g{n}"] = gsb

            return f

        # expert FFN units (resident wr1 / wr2)
        def mk_m1(n, i, h_sb):
            def f():
                xT = xe_t[n]
                ps_a = pmm.tile([128, 512], dt.float32, tag="mm")
                for k in range(KD):
                    nc.tensor.matmul(
                        ps_a[:],
                        lhsT=wr1[k][:, i * 128 : (i + 1) * 128],
                        rhs=xT[:, k * 512 : (k + 1) * 512],
                        start=(k == 0),
                        stop=(k == KD - 1),
                    )
                ps_g = pmm.tile([128, 512], dt.float32, tag="mm")
                for k in range(KD):
                    nc.tensor.matmul(
                        ps_g[:],
                        lhsT=wr1[k][:, (16 + i) * 128 : (17 + i) * 128],
                        rhs=xT[:, k * 512 : (k + 1) * 512],
                        start=(k == 0),
                        stop=(k == KD - 1),
                    )
                sl = sb1.tile([128, 512], dt.bfloat16, tag=f"silu{i % 2}")
                nc.scalar.activation(sl[:], ps_g[:], AF.Silu)
                nc.vector.tensor_mul(
                    h_sb[:, i * 512 : (i + 1) * 512], ps_a[:], sl[:]
                )

            return f

        def mk_m2(wr2_l, h_sb, out_dram, out_col, gate_sb, mo):
            def f():
                ps2 = pmm.tile([128, 512], dt.float32, tag="mm")
                for k2 in range(KI):
                    nc.tensor.matmul(
                        ps2[:],
                        lhsT=wr2_l[k2][:, mo * 128 : (mo + 1) * 128],
                        rhs=h_sb[:, k2 * 512 : (k2 + 1) * 512],
                        start=(k2 == 0),
                        stop=(k2 == KI - 1),
                    )
                yo = sb1.tile([128, 512], dt.bfloat16, tag=f"yo{mo % 2}")
                if gate_sb is not None:
                    nc.vector.tensor_mul(yo[:], ps2[:], gate_sb[:])
                else:
                    nc.scalar.activation(yo[:], ps2[:], AF.Copy)
                nc.sync.dma_start(
                    out_dram[mo * 128 : (mo + 1) * 128, out_col : out_col + 512],
                    yo[:],
                )

            return f

        # ---------------- emission schedule ----------------
        # routing units, in dependency order; dispatch 0/1 right after
        # compact so their ring positions precede all later DMA traffic
        routing_units = [rt_init, rt_sig]
        for it in range(BISECT_ITERS):
            routing_units.append(mk_bisect(it))
        routing_units.append(rt_gates)
        routing_units.append(rt_compact)
        routing_units.append(mk_dispatch(0))
        routing_units.append(mk_dispatch(1))

        # shared W2 woven into phase 1 (needed when m1 ends ~100us); expert W1
        # waits for phase 2, keeping 8.4MB out of the phase-1 DMA path
        wload_units = [
            mk_wload(wrs2, k2, t["ws2_b"], DIM, "wr2s_") for k2 in range(KI)
        ]

        # phase 1: shared m1 (32 half-units); input/router DMAs first, the
        # routing chain woven 3 per half-unit from half-unit 6 (its stalls
        # hide inside the DMA-bound m1 pace)
        w1s_dma(0)
        xsT_dma(0)
        mk_router(0)()
        xsT_dma(1)
        mk_router(1)()
        rt_collective()

        ru = iter(routing_units)
        wl = iter(wload_units)
        half_units = []
        for i in range(16):
            half_units.append(sh_m1_half(i, 0))
            half_units.append(sh_m1_half(i, 1))
        emitted = 0
        for ui, u in enumerate(half_units):
            u()
            nxt_w = next(wl, None)
            if nxt_w is not None:
                nxt_w()
            if ui >= 6:
                want = (ui - 5) * 3
                while emitted < want:
                    nxt = next(ru, None)
                    if nxt is None:
                        break
                    nxt()
                    emitted += 1
        for r in ru:
            r()
        for w in wl:
            w()

        # phase 2: shared m2 batch 0 only (8 units) with expert W1 loads
        # woven in; batch 1 is deferred into the expert phase so the PE
        # reaches the expert chunks ~35us earlier and batch-1's output
        # writes leave the pre-gather fence window
        wl1 = iter(
            [mk_wload(wr1, k, t["w1_b"], I2, "wr1e_") for k in range(KD)]
        )
        for mo in range(KD):
            mk_m2(wrs2, h_sh[0], t["out_shared"], 0, None, mo)()
            nxt_w = next(wl1, None)
            if nxt_w is not None:
                nxt_w()

        # phase 3: expert FFN, 4 chunks of 512 slots. Expert W2 (aliasing the
        # wrs2 tiles) is woven into chunk 0's m1 so its in-flight window sits
        # in an otherwise DMA-quiet stretch, after the dispatch gathers.
        for n in range(SLOTS // 512):
            hnew = sb1.tile([128, KI * 512], dt.bfloat16, tag=f"h_sb{n % 2}")
            for i in range(16):
                mk_m1(n, i, hnew)()
            if n == 0:
                # shared m2 batch 1 must fully read the wrs2 tiles before
                # the expert W2 loads overwrite them
                for mo in range(KD):
                    mk_m2(wrs2, h_sh[1], t["out_shared"], 512, None, mo)()
                for k2 in range(KI):
                    mk_wload(wr2, k2, t["w2_b"], DIM, "wr2s_")()
            for mo in range(KD):
                mk_m2(wr2, hnew, t["out_routed"], n * 512, gate_t[f"g{n}"], mo)()
            if n + 2 < SLOTS // 512:
                mk_dispatch(n + 2)()


# ======================= host side =======================

_CACHED_NC = None


def _get_nc():
    global _CACHED_NC
    if _CACHED_NC is None:
        _CACHED_NC = build_nc()
    return _CACHED_NC


def make_in_maps(inputs):
    hs_flat = np.ascontiguousarray(
        np.asarray(inputs["hidden_states"], dtype=np.float32).reshape(T, DIM)
    )
    hsu_flat = np.ascontiguousarray(
        np.asarray(inputs["hidden_states_unmodulated"], dtype=np.float32).reshape(
            T, DIM
        )
    )
    ts = np.asarray(inputs["timestep"], dtype=np.float32)
    Wg = np.asarray(inputs["Wg"], dtype=np.float32)
    W1 = np.asarray(inputs["W1"], dtype=np.float32)
    W2 = np.asarray(inputs["W2"], dtype=np.float32)
    Ws1 = np.asarray(inputs["Ws1"], dtype=np.float32)
    Ws2 = np.asarray(inputs["Ws2"], dtype=np.float32)

    lt16 = np.triu(np.ones((16, 16), np.float32), 1)  # lhsT[k,m]=1 iff k<m
    b2 = np.zeros((16, BS), np.float32)
    b2[:8, 0] = 1.0
    b2[8:, 1] = 1.0
    # partition layout: p = r*16 + e*2 + c  (r = source core, e = expert,
    # c = 512-token half of the core's shard)
    p = np.arange(128)
    pb = p // 64  # batch  (r//4)
    pe = (p % 16) // 2  # expert
    rsmat = ((pb[:, None] == pb[None, :]) & (pe[:, None] == pe[None, :])).astype(
        np.float32
    )
    # dsel[p, j]: p belongs to token-group j = r*2 + c (sum over experts)
    j = np.arange(16)
    dsel = ((p[:, None] // 16 == j[None, :] // 2) & (p[:, None] % 2 == j[None, :] % 2)
            ).astype(np.float32)
    ones128 = np.ones((1, 128), np.float32)
    jj = np.arange(16)[:, None]
    tt = np.arange(512)[None, :]
    iota_tid = (jj * 512 + tt).astype(np.uint16)
    # router: transposed Wg (hsu half) and host-computed timestep bias
    wgt = np.ascontiguousarray(Wg[:, DIM:].T)  # [DIM, E]
    bias_all = ts @ Wg[:, :DIM].T  # [BS, E]

    import ml_dtypes

    bf16 = ml_dtypes.bfloat16
    hs_b = hs_flat.astype(bf16)
    W1_b = W1.astype(bf16)
    W2_b = W2.astype(bf16)
    # shared W1 packed (a_i | g_i) column pairs
    a = Ws1[:, :INNER].reshape(DIM, 16, 128)
    g = Ws1[:, INNER:].reshape(DIM, 16, 128)
    Ws1p = np.ascontiguousarray(
        np.concatenate([a[:, :, None, :], g[:, :, None, :]], axis=2).reshape(DIM, I2)
    ).astype(bf16)
    Ws2_b = np.ascontiguousarray(Ws2.astype(bf16))
    in_maps = []
    for c in range(NCORES):
        # extract my expert's 16 rows in (b-major, chunk) order:
        # j = r*2 + cc  ->  partition (j//2)*16 + c*2 + (j%2)
        esel = np.zeros((128, 16), np.float32)
        for jx in range(16):
            esel[(jx // 2) * 16 + c * 2 + (jx % 2), jx] = 1.0
        in_maps.append(
            {
                "hs_b": hs_b,
                "hs_shT_b": np.ascontiguousarray(hs_b[c * TSH : (c + 1) * TSH].T),
                "hsuT_sh": np.ascontiguousarray(
                    hsu_flat[c * TSH : (c + 1) * TSH].T
                ),
                "wgt": wgt,
                "bias_mine": np.ascontiguousarray(
                    bias_all[c // 4].reshape(E, 1)
                ),
                "w1_b": np.ascontiguousarray(W1_b[c]),
                "w1p_b": Ws1p,
                "w2_b": np.ascontiguousarray(W2_b[c]),
                "ws2_b": Ws2_b,
                "esel": esel,
                "dsel": dsel,
                "b2": b2,
                "lt16": lt16,
                "rsmat": rsmat,
                "ones128": ones128,
                "iota_tid": iota_tid,
            }
        )
    return in_maps


def combine(results):
    out = np.empty((T, DIM), np.float32)
    for c in range(NCORES):
        out[c * TSH : (c + 1) * TSH] = results[c]["out_shared"].T.astype(np.float32)
    for c in range(NCORES):
        idx = results[c]["out_idx"].reshape(SLOTS)
        out[idx] += results[c]["out_routed"].T.astype(np.float32)
    return out.reshape(BS, SLEN, DIM)


def kernel(**inputs):
    nc = _get_nc()
    in_maps = make_in_maps(inputs)
    res = run_bass_kernel_spmd(nc, in_maps, list(range(NCORES))).results
    return combine(res)


if __name__ == "__main__":
    nc = build_nc()
    print("build ok:", len(nc.inst_map), "instructions")
